# revision 3
# baseline (speedup 1.0000x reference)
"""DeltaNet attention (per-chunk delta-rule scan) as a Trainium2 Bass kernel.

Shapes (hardcoded from the problem spec):
  x [B=8, T=4096, D=512], H=4 heads, head_dim d=128, dv=256, chunk C=64.

Math: within each 64-token chunk the recurrence
    S_t = (1-b_t) S_{t-1} + b_t k_t v_t^T ;  o_t = q_t^T S_t   (S reset per chunk)
unrolls to masked intra-chunk attention:
    o_t = sum_{s<=t} [qn_t . kn_s] * b_s * exp(l_t - l_s) * v_s,
    l_t = sum_{r<=t} log(1-b_r),  qn/kn = rmsnorm'd q/k.
All per-token factors (rms scale, b_s, decay exp(l - l_mid)) fold into q/k as
per-(token,head) scalars, referenced to the chunk midpoint for fp32 safety.

Sharding: data-parallel over B across the 8 NeuronCores (SPMD, no collectives).
"""
import numpy as np

import concourse.bacc as bacc
import concourse.mybir as mybir
from concourse import tile
from concourse.bass_utils import run_bass_kernel_spmd

F32 = mybir.dt.float32
F32R = mybir.dt.float32r
BF16 = mybir.dt.bfloat16
AF = mybir.ActivationFunctionType
MUL = mybir.AluOpType.mult

B, T, D = 8, 4096, 512
H, C = 4, 64
d = 128          # head dim
dv = 256         # value head dim
P = 128          # tokens per tile (2 chunks)
NT = T // P      # 32 tiles
MID = 31         # decay reference index within a chunk
RMS_EPS = 1.1920929e-07

# dtype config: proj matmuls (x->qkv, out-proj) and scan matmuls (A0T, OT)
PROJ_DT = F32R
SCAN_DT = F32


def _consts():
    lidx = np.arange(C)
    r_le_t = (lidx[:, None] <= lidx[None, :]).astype(np.float32)   # [r, t]
    r_le_m = (lidx[:, None] <= MID).astype(np.float32) * np.ones((1, C), np.float32)
    blk = np.zeros((P, P), np.float32)
    udq = np.zeros((P, P), np.float32)
    for c in range(P // C):
        sl = slice(c * C, (c + 1) * C)
        blk[sl, sl] = r_le_t
        udq[sl, sl] = r_le_m - r_le_t
    maskt = blk  # mask[s, t] = 1 iff s <= t within the same chunk
    return udq, -udq, maskt


def build_nc(proj_dt=PROJ_DT, scan_dt=SCAN_DT, rep=1, nt=NT):
    nc = bacc.Bacc("TRN2", target_bir_lowering=False, debug=False, num_devices=8)

    x_d = nc.dram_tensor("x", [T, D], proj_dt, kind="ExternalInput")
    wq_d = nc.dram_tensor("Wq", [D, H * d], proj_dt, kind="ExternalInput")
    wk_d = nc.dram_tensor("Wk", [D, H * d], proj_dt, kind="ExternalInput")
    wv_d = nc.dram_tensor("Wv", [D, H * dv], proj_dt, kind="ExternalInput")
    wb_d = nc.dram_tensor("Wbeta", [D, H], proj_dt, kind="ExternalInput")
    wp_d = nc.dram_tensor("Wproj", [H * dv, D], proj_dt, kind="ExternalInput")
    idp_d = nc.dram_tensor("identp", [P, P], proj_dt, kind="ExternalInput")
    id32_d = nc.dram_tensor("ident32", [P, P], F32, kind="ExternalInput")
    udq_d = nc.dram_tensor("udq", [P, P], F32, kind="ExternalInput")
    udk_d = nc.dram_tensor("udk", [P, P], F32, kind="ExternalInput")
    mask_d = nc.dram_tensor("maskt", [P, P], F32, kind="ExternalInput")
    y_d = nc.dram_tensor("y", [T, D], F32, kind="ExternalOutput")

    with tile.TileContext(nc) as tc:
        with (
            tc.tile_pool(name="wpool", bufs=1) as wp,
            tc.tile_pool(name="sbuf", bufs=2) as sb,
            tc.tile_pool(name="tiny", bufs=2) as tb,
            tc.tile_pool(name="psb", bufs=6, space="PSUM") as psb,
            tc.tile_pool(name="pst", bufs=2, space="PSUM") as pst,
        ):
            # --- resident weights / consts ---
            wq_sb = wp.tile([P, 4, 512], proj_dt)
            wk_sb = wp.tile([P, 4, 512], proj_dt)
            wv_sb = wp.tile([P, 4, 1024], proj_dt)
            wb_sb = wp.tile([P, 4, 4], proj_dt)
            wp_sb = wp.tile([P, 8, 512], proj_dt)
            for j in range(4):
                nc.sync.dma_start(out=wq_sb[:, j, :], in_=wq_d[j * P:(j + 1) * P, :])
                nc.sync.dma_start(out=wk_sb[:, j, :], in_=wk_d[j * P:(j + 1) * P, :])
                nc.sync.dma_start(out=wv_sb[:, j, :], in_=wv_d[j * P:(j + 1) * P, :])
                nc.sync.dma_start(out=wb_sb[:, j, :], in_=wb_d[j * P:(j + 1) * P, :])
            for j in range(8):
                nc.sync.dma_start(out=wp_sb[:, j, :], in_=wp_d[j * P:(j + 1) * P, :])
            eps_sb = wp.tile([P, 1], F32)
            nc.gpsimd.memset(eps_sb[:], RMS_EPS)
            idp_sb = wp.tile([P, P], proj_dt)
            id32_sb = wp.tile([P, P], F32)
            udq_sb = wp.tile([P, P], F32)
            udk_sb = wp.tile([P, P], F32)
            mask_sb = wp.tile([P, P], F32)
            nc.sync.dma_start(out=idp_sb[:], in_=idp_d[:])
            nc.sync.dma_start(out=id32_sb[:], in_=id32_d[:])
            nc.sync.dma_start(out=udq_sb[:], in_=udq_d[:])
            nc.sync.dma_start(out=udk_sb[:], in_=udk_d[:])
            nc.sync.dma_start(out=mask_sb[:], in_=mask_d[:])

            for _ in range(rep):
                for it in range(nt):
                    t0 = it * P
                    # ---- load x tile, transpose to feature-major ----
                    x_sb = sb.tile([P, 512], proj_dt, tag="x")
                    nc.sync.dma_start(out=x_sb[:], in_=x_d[t0:t0 + P, :])
                    xt_ps = psb.tile([P, 512], proj_dt, tag="ps512")
                    for j in range(4):
                        nc.tensor.transpose(
                            xt_ps[:, j * P:(j + 1) * P],
                            x_sb[:, j * P:(j + 1) * P], idp_sb[:])
                    xt_sb = sb.tile([P, 4, P], proj_dt, tag="xt")
                    nc.scalar.copy(xt_sb[:], xt_ps[:].rearrange("p (j t) -> p j t", j=4))

                    # ---- projections ----
                    q_ps = psb.tile([P, 512], F32, tag="ps512")
                    k_ps = psb.tile([P, 512], F32, tag="ps512")
                    v0_ps = psb.tile([P, 512], F32, tag="ps512")
                    v1_ps = psb.tile([P, 512], F32, tag="ps512")
                    bl_ps = pst.tile([P, 4], F32, tag="pstiny")
                    for j in range(4):
                        nc.tensor.matmul(q_ps[:], xt_sb[:, j, :], wq_sb[:, j, :],
                                         start=(j == 0), stop=(j == 3))
                    for j in range(4):
                        nc.tensor.matmul(k_ps[:], xt_sb[:, j, :], wk_sb[:, j, :],
                                         start=(j == 0), stop=(j == 3))
                    for j in range(4):
                        nc.tensor.matmul(v0_ps[:], xt_sb[:, j, :], wv_sb[:, j, 0:512],
                                         start=(j == 0), stop=(j == 3))
                    for j in range(4):
                        nc.tensor.matmul(v1_ps[:], xt_sb[:, j, :], wv_sb[:, j, 512:1024],
                                         start=(j == 0), stop=(j == 3))
                    for j in range(4):
                        nc.tensor.matmul(bl_ps[:], xt_sb[:, j, :], wb_sb[:, j, :],
                                         start=(j == 0), stop=(j == 3))

                    # ---- beta / decay chain (all tiny [P,4]) ----
                    e_sb = tb.tile([P, 4], F32, tag="e")
                    nc.scalar.activation(e_sb[:], bl_ps[:], AF.Exp)
                    sp1 = tb.tile([P, 4], F32, tag="sp1")      # 1 + e^z
                    nc.vector.tensor_scalar_add(sp1[:], e_sb[:], 1.0)
                    sp_sb = tb.tile([P, 4], F32, tag="sp")     # softplus(z)
                    nc.scalar.activation(sp_sb[:], sp1[:], AF.Ln)
                    rec = tb.tile([P, 4], F32, tag="rec")
                    nc.vector.reciprocal(rec[:], sp1[:])
                    beta = tb.tile([P, 4], F32, tag="beta")    # sigmoid(z)
                    nc.vector.tensor_tensor(out=beta[:], in0=e_sb[:], in1=rec[:], op=MUL)

                    dlq_ps = pst.tile([P, 4], F32, tag="pstiny")
                    dlk_ps = pst.tile([P, 4], F32, tag="pstiny")
                    nc.tensor.matmul(dlq_ps[:], udq_sb[:], sp_sb[:], start=True, stop=True)
                    nc.tensor.matmul(dlk_ps[:], udk_sb[:], sp_sb[:], start=True, stop=True)
                    rowdec = tb.tile([P, 4], F32, tag="rowdec")
                    coldec = tb.tile([P, 4], F32, tag="coldec")
                    nc.scalar.activation(rowdec[:], dlq_ps[:], AF.Exp)
                    nc.scalar.activation(coldec[:], dlk_ps[:], AF.Exp)

                    # ---- rmsnorm scales: g = exp(-0.5*ln(ssq/128 + eps)) ----
                    sqs = sb.tile([P, 512], F32, tag="sqs")
                    ssq_q = tb.tile([P, 4], F32, tag="ssq_q")
                    ssq_k = tb.tile([P, 4], F32, tag="ssq_k")
                    for h in range(4):
                        nc.scalar.activation(sqs[:, h * P:(h + 1) * P],
                                             q_ps[:, h * P:(h + 1) * P], AF.Square,
                                             accum_out=ssq_q[:, h:h + 1])
                    for h in range(4):
                        nc.scalar.activation(sqs[:, h * P:(h + 1) * P],
                                             k_ps[:, h * P:(h + 1) * P], AF.Square,
                                             accum_out=ssq_k[:, h:h + 1])
                    lnq = tb.tile([P, 4], F32, tag="lnq")
                    lnk = tb.tile([P, 4], F32, tag="lnk")
                    nc.scalar.activation(lnq[:], ssq_q[:], AF.Ln, scale=1.0 / d, bias=eps_sb[:])
                    nc.scalar.activation(lnk[:], ssq_k[:], AF.Ln, scale=1.0 / d, bias=eps_sb[:])
                    gq = tb.tile([P, 4], F32, tag="gq")
                    gk = tb.tile([P, 4], F32, tag="gk")
                    nc.scalar.activation(gq[:], lnq[:], AF.Exp, scale=-0.5)
                    nc.scalar.activation(gk[:], lnk[:], AF.Exp, scale=-0.5)

                    qscale = tb.tile([P, 4], F32, tag="qscale")
                    nc.vector.tensor_tensor(out=qscale[:], in0=gq[:], in1=rowdec[:], op=MUL)
                    kt1 = tb.tile([P, 4], F32, tag="kt1")
                    nc.vector.tensor_tensor(out=kt1[:], in0=gk[:], in1=beta[:], op=MUL)
                    kscale = tb.tile([P, 4], F32, tag="kscale")
                    nc.vector.tensor_tensor(out=kscale[:], in0=kt1[:], in1=coldec[:], op=MUL)

                    # ---- scaled evac of q/k (fold all per-token factors) ----
                    q_sb = sb.tile([P, 4, P], F32, tag="q")
                    k_sb = sb.tile([P, 4, P], F32, tag="k")
                    nc.vector.tensor_tensor(
                        out=q_sb[:], in0=q_ps[:].rearrange("p (h t) -> p h t", h=4),
                        in1=qscale[:].unsqueeze(-1).broadcast_to([P, 4, P]), op=MUL)
                    nc.vector.tensor_tensor(
                        out=k_sb[:], in0=k_ps[:].rearrange("p (h t) -> p h t", h=4),
                        in1=kscale[:].unsqueeze(-1).broadcast_to([P, 4, P]), op=MUL)
                    v_sb = sb.tile([P, 1024], scan_dt, tag="v")
                    nc.scalar.copy(v_sb[:, 0:512], v0_ps[:])
                    nc.scalar.copy(v_sb[:, 512:1024], v1_ps[:])

                    # ---- transpose q/k to feature-major ----
                    qt_ps = psb.tile([P, 512], F32, tag="ps512")
                    kt_ps = psb.tile([P, 512], F32, tag="ps512")
                    for h in range(4):
                        nc.tensor.transpose(qt_ps[:, h * P:(h + 1) * P],
                                            q_sb[:, h, :], id32_sb[:])
                    for h in range(4):
                        nc.tensor.transpose(kt_ps[:, h * P:(h + 1) * P],
                                            k_sb[:, h, :], id32_sb[:])
                    qt_sb = sb.tile([P, 4, P], scan_dt, tag="qt")
                    kt_sb = sb.tile([P, 4, P], scan_dt, tag="kt")
                    nc.vector.tensor_copy(qt_sb[:], qt_ps[:].rearrange("p (h t) -> p h t", h=4))
                    nc.vector.tensor_copy(kt_sb[:], kt_ps[:].rearrange("p (h t) -> p h t", h=4))

                    # ---- A0T = k'^T q' per head; mask at evac ----
                    a_ps = psb.tile([P, 512], F32, tag="ps512")
                    for h in range(4):
                        nc.tensor.matmul(a_ps[:, h * P:(h + 1) * P],
                                         kt_sb[:, h, :], qt_sb[:, h, :],
                                         start=True, stop=True)
                    at_sb = sb.tile([P, 4, P], scan_dt, tag="at")
                    nc.vector.tensor_tensor(
                        out=at_sb[:], in0=a_ps[:].rearrange("p (h t) -> p h t", h=4),
                        in1=mask_sb[:].unsqueeze(1).broadcast_to([P, 4, P]), op=MUL)

                    # ---- OT = V'^T A^T  (dv split in halves) ----
                    ot0_ps = psb.tile([P, 512], F32, tag="ps512")
                    ot1_ps = psb.tile([P, 512], F32, tag="ps512")
                    for h in range(4):
                        nc.tensor.matmul(ot0_ps[:, h * P:(h + 1) * P],
                                         v_sb[:, h * dv:h * dv + P], at_sb[:, h, :],
                                         start=True, stop=True)
                    for h in range(4):
                        nc.tensor.matmul(ot1_ps[:, h * P:(h + 1) * P],
                                         v_sb[:, h * dv + P:h * dv + dv], at_sb[:, h, :],
                                         start=True, stop=True)
                    ot_sb = sb.tile([P, 8, P], proj_dt, tag="ot")
                    otv = ot_sb[:].rearrange("p (h w) t -> p h w t", w=2)
                    nc.scalar.copy(otv[:, :, 0, :], ot0_ps[:].rearrange("p (h t) -> p h t", h=4))
                    nc.scalar.copy(otv[:, :, 1, :], ot1_ps[:].rearrange("p (h t) -> p h t", h=4))

                    # ---- output projection ----
                    out_ps = psb.tile([P, 512], F32, tag="ps512")
                    for j in range(8):
                        nc.tensor.matmul(out_ps[:], ot_sb[:, j, :], wp_sb[:, j, :],
                                         start=(j == 0), stop=(j == 7))
                    out_sb = sb.tile([P, 512], F32, tag="out")
                    nc.vector.tensor_copy(out_sb[:], out_ps[:])
                    nc.sync.dma_start(out=y_d[t0:t0 + P, :], in_=out_sb[:])

    nc.compile()
    return nc


_NC_CACHE = {}


def _get_nc():
    key = (str(PROJ_DT), str(SCAN_DT))
    if key not in _NC_CACHE:
        _NC_CACHE[key] = build_nc()
    return _NC_CACHE[key]


def make_in_maps(x, Wq, Wk, Wv, Wbeta, Wproj):
    udq, udk, maskt = _consts()
    ident = np.eye(P, dtype=np.float32)
    base = {
        "Wq": np.ascontiguousarray(Wq, np.float32),
        "Wk": np.ascontiguousarray(Wk, np.float32),
        "Wv": np.ascontiguousarray(Wv, np.float32),
        "Wbeta": np.ascontiguousarray(Wbeta, np.float32),
        "Wproj": np.ascontiguousarray(Wproj, np.float32),
        "identp": ident, "ident32": ident,
        "udq": udq, "udk": udk, "maskt": maskt,
    }
    return [dict(base, x=np.ascontiguousarray(x[b], np.float32)) for b in range(B)]


def kernel(x, ve=None, cos_sin=None, Wq=None, Wk=None, Wv=None, Wbeta=None,
           Wproj=None, window_size=None, **_ignored):
    x = np.asarray(x, np.float32)
    nc = _get_nc()
    in_maps = make_in_maps(x, Wq, Wk, Wv, Wbeta, Wproj)
    res = run_bass_kernel_spmd(nc, in_maps, core_ids=list(range(B)))
    return np.stack([res.results[b]["y"] for b in range(B)]).astype(np.float32)
